# revision 31
# baseline (speedup 1.0000x reference)
"""Trainium2 Bass kernel for ContralateralInteractionModule (v3, bf16).

Full computation (per sample b, C=128 channels, N=32768 spatial):
    rf   = flip(right, h)
    kv   = W @ concat(left, rf) + bias                      # [C, N]
    A_l  = softmax(left @ kv.T / sqrt(N))                   # [C, C]
    A_r  = softmax(rf   @ kv.T / sqrt(N))
    e_l  = A_l @ kv + left ;  e_r = A_r @ kv + rf
    gate = sigmoid(conv1d_k5(mean_N(e)))  (per side, ECA)
    out_l = e_l * gate_l ; out_r = flip(e_r * gate_r, h)

Sharding: 8 cores = 2 samples x 4 spatial quarters (n_loc = 8192).
Each core accumulates partial Gram matrices, computes partial
(pre-softmax) logits from them, and one AllReduce per 4-core group sums
the logits; softmax/gate are computed redundantly; each core emits its
own output quarter.

v3 changes vs v1 (the f32/f32r kernel):
  - All streamed tensors are bf16: inputs, transposed Gram operands, kv,
    outputs. Host converts f32->bf16 before DMA and upcasts after.
    Halves HBM traffic and makes every PE matmul 1 cycle/row.
  - Spatial sums (xsum_l/r), the rank-1 logit bias term t1 = xsum x bias,
    the ECA kv-mean vector kvsum, and the ECA input-mean conv term hb are
    all precomputed on HOST (pure functions of the inputs): removes the
    on-device ACT/DVE reduction passes entirely.
  - The AllReduce carries pre-reduced partial logits [P, 256] f32
    (the W-projection of the partial Gram runs pre-AR per core), not the
    [P, 520] Gram+sums payload.
  - Single activation table (exp_and_friends): sigmoid is computed as
    1/(1+exp(-x)) so Exp/Identity/Copy cover every ACT op -> one
    LoadActFuncSet instead of four.
  - softmax normalization (recip) and ECA gate fold into the matmul
    stationary: exps_s = (gate*recip) . exps; the kv bias folds into kv
    during its PSUM evacuation (kv' = kv + bias). Phase B is then ONE
    matmul per block plus a fused scalar_tensor_tensor evacuation
    out = x*gate + psum  (== gate.(attn@kv_biased + x)).
"""

import numpy as np
import ml_dtypes

import concourse.bacc as bacc
import concourse.bass as bass
import concourse.tile as tile
import concourse.mybir as mybir
from concourse.bass_utils import run_bass_kernel_spmd
from concourse.masks import make_identity

P = 128                    # channels == partitions
N_GLOBAL = 32768           # 32*32*32 spatial
N_CORES = 8
N_SPLIT = 4                # spatial quarters per sample
NLOC = N_GLOBAL // N_SPLIT # 8192 per core
BLK = 512                  # block width (free dim)
NBLK = NLOC // BLK         # 16
XBS = NBLK - 4             # first XBAR-transposed block
SM_SCALE = 1.0 / float(np.sqrt(np.float32(N_GLOBAL)))
F32 = mybir.dt.float32
BF16 = mybir.dt.bfloat16
NPBF16 = ml_dtypes.bfloat16

REPLICA_GROUPS = [[0, 1, 2, 3], [4, 5, 6, 7]]
AR_W = 2 * P  # [ltp_l | ltp_r]

_CACHE: dict = {}


def _build_nc(repeat: int = 1, single: bool = False):
    nc = bacc.Bacc("TRN2", target_bir_lowering=False, debug=False,
                   num_devices=1 if single else N_CORES)
    nc._single_core_variant = single

    xl_d = nc.dram_tensor("xl", [P, NLOC], BF16, kind="ExternalInput").ap()
    xr_d = nc.dram_tensor("xr", [P, NLOC], BF16, kind="ExternalInput").ap()
    # packed weights: wpk = [w0t|w1t|t1l|t1r|kvs] bf16, fpk = [bt|kvb|mhb] f32
    wpk_d = nc.dram_tensor("wpk", [P, 4 * P + 1], BF16,
                           kind="ExternalInput").ap()
    fpk_d = nc.dram_tensor("fpk", [P, P + 3], F32,
                           kind="ExternalInput").ap()
    ol_d = nc.dram_tensor("ol", [P, NLOC], BF16, kind="ExternalOutput").ap()
    or_d = nc.dram_tensor("orr", [P, NLOC], BF16, kind="ExternalOutput").ap()

    with tile.TileContext(nc) as tc:
        with (
            tc.tile_pool(name="persist", bufs=1) as pp,
            tc.tile_pool(name="dram", bufs=1, space="DRAM") as dram,
        ):
            g = {}
            g["xl"] = pp.tile([P, NLOC], BF16, tag="xl", name="xl")
            g["xr"] = pp.tile([P, NLOC], BF16, tag="xr", name="xr")
            g["kv"] = pp.tile([P, NLOC], BF16, tag="kv", name="kv")
            wpk = pp.tile([P, 4 * P + 1], BF16, tag="wpk", name="wpk")
            fpk = pp.tile([P, P + 3], F32, tag="fpk", name="fpk")
            g["wpk"], g["fpk"] = wpk, fpk
            g["w0t"] = wpk[:, 0:P]
            g["w1t"] = wpk[:, P:2 * P]
            g["t1l"] = wpk[:, 2 * P:3 * P]
            g["t1r"] = wpk[:, 3 * P:4 * P]
            g["kvs"] = wpk[:, 4 * P:4 * P + 1]
            g["bt"] = fpk[:, 0:P]
            g["kvb"] = fpk[:, P:P + 1]
            g["mhbl"] = fpk[:, P + 1:P + 2]
            g["mhbr"] = fpk[:, P + 2:P + 3]
            ident = pp.tile([P, P], F32, tag="ident", name="ident")
            identr = pp.tile([P, P], BF16, tag="identr", name="identr")
            g["ident"] = ident
            g["identr"] = identr
            g["ar_in"] = pp.tile([P, AR_W], BF16, tag="ar_in", name="ar_in")
            g["ar_out"] = pp.tile([P, AR_W], BF16, tag="ar_out",
                                  name="ar_out")
            g["cc_in"] = dram.tile([P, AR_W], BF16, name="cc_in")
            g["cc_out"] = dram.tile([P, AR_W], BF16, name="cc_out")

            make_identity(nc, identr[:])
            nc.vector.tensor_copy(ident[:], identr[:])

            for _rep in range(repeat):
                _build_iter(nc, tc, g, xl_d, xr_d, wpk_d, fpk_d,
                            ol_d, or_d)

    nc.compile()
    return nc


def _build_iter(nc, tc, g, xl_d, xr_d, wpk_d, fpk_d, ol_d, or_d):
    xl, xr, kv = g["xl"], g["xr"], g["kv"]
    identr = g["identr"]

    # ---------------- Phase A: Gram accumulation ----------------
    # Inputs stream in chunks. Per 512-block: transpose ql/qr 128-chunks
    # on PE into packed [qlT|qrT|qlT|qrT] tiles; accumulate g1 = [A|B]
    # (lhsT=qlT, rhs 256-wide) and g2 = C (lhsT=qrT, rhs 128-wide).
    with (
        tc.tile_pool(name="psG", bufs=1, space="PSUM") as psG,
        tc.tile_pool(name="psT", bufs=4, space="PSUM") as psT,
        tc.tile_pool(name="sbT", bufs=6) as sbT,
    ):
        g12 = psG.tile([P, 3 * P], F32, tag="g12", name="g12")
        g1 = g12[:, 0:2 * P]
        g2 = g12[:, 2 * P:3 * P]

        bounds = [0, 1024, 2048, 4096, 8192]
        for ch in range(4):
            cs_ = slice(bounds[ch], bounds[ch + 1])
            nc.sync.dma_start(out=xl[:, cs_], in_=xl_d[:, cs_])
            nc.sync.dma_start(out=xr[:, cs_], in_=xr_d[:, cs_])
            if ch == 0:
                # weights ride behind the first input chunk
                nc.sync.dma_start(out=g["wpk"][:], in_=wpk_d)
                nc.sync.dma_start(out=g["fpk"][:], in_=fpk_d)

            for b in range(bounds[ch] // BLK, bounds[ch + 1] // BLK):
                bs = slice(b * BLK, (b + 1) * BLK)
                if b >= XBS:
                    # XBAR path: DMA per-128-chunk transposes (SBUF->SBUF),
                    # Gram as 3 separate 128-wide accumulations into g12b
                    qt2 = sbT.tile([P, 8, P], BF16, tag="trs", name="qt2")
                    nc.sync.dma_start_transpose(qt2[:, 0:4, :], xl[:, bs])
                    nc.sync.dma_start_transpose(qt2[:, 4:8, :], xr[:, bs])
                    # join g12's open bank group: start=False always
                    # (PE path's ci==0 start executes first in PE order)
                    for c4 in range(4):
                        last = b == NBLK - 1 and c4 == 3
                        ql = qt2[:, c4, :]
                        qr = qt2[:, 4 + c4, :]
                        nc.tensor.matmul(g12[:, 0:P], ql, ql,
                                         start=False, stop=last,
                                         skip_group_check=True)
                        nc.tensor.matmul(g12[:, P:2 * P], ql, qr,
                                         start=False, stop=last,
                                         skip_group_check=True)
                        nc.tensor.matmul(g12[:, 2 * P:3 * P], qr, qr,
                                         start=False, stop=last,
                                         skip_group_check=True)
                    continue
                qp = psT.tile([P, 2 * BLK], BF16, tag="trp", name="qp")
                for c4 in range(4):
                    cs = slice(b * BLK + c4 * P, b * BLK + (c4 + 1) * P)
                    qo = c4 * 2 * P
                    nc.tensor.transpose(qp[:, qo:qo + P], xl[:, cs],
                                        identr[:])
                    nc.tensor.transpose(qp[:, qo + P:qo + 2 * P],
                                        xr[:, cs], identr[:])
                qt = sbT.tile([P, 2 * BLK], BF16, tag="trs", name="qt")
                if b % 8 < 5:
                    nc.vector.tensor_copy(qt[:], qp[:])
                else:
                    nc.scalar.copy(qt[:], qp[:])

                for c4 in range(4):
                    ci = b * 4 + c4
                    qo = c4 * 2 * P
                    first = ci == 0
                    nc.tensor.matmul(g1, qt[:, qo:qo + P],
                                     qt[:, qo:qo + 2 * P],
                                     start=first, stop=False,
                                     skip_group_check=True)
                    nc.tensor.matmul(g2, qt[:, qo + P:qo + 2 * P],
                                     qt[:, qo + P:qo + 2 * P],
                                     start=False, stop=False,
                                     skip_group_check=True)

        # ---------------- pre-AR: partial logits ----------------
        # gsb = [A | B | C] bf16; ltp_s = w0t.T@G_u + w1t.T@G_l where
        # side l uses (A, B^T), side r uses (B, C).
        gsb = sbT.tile([P, 3 * P], BF16, tag="gsb", name="gsb")
        nc.scalar.copy(gsb[:, 0:2 * P], g1)
        nc.vector.tensor_copy(gsb[:, 2 * P:3 * P], g2)

        tbp = psG.tile([P, P], BF16, tag="tbp", name="tbp")
        nc.tensor.transpose(tbp[:], gsb[:, P:2 * P], identr[:])
        tbsb = sbT.tile([P, P], BF16, tag="tbsb", name="tbsb")
        nc.vector.tensor_copy(tbsb[:], tbp[:])

        ltps = psG.tile([P, 2 * P], F32, tag="ltps", name="ltps")
        ltpl = ltps[:, 0:P]
        ltpr = ltps[:, P:2 * P]
        nc.tensor.matmul(ltpl, g["w0t"], gsb[:, 0:P],
                         start=True, stop=False, skip_group_check=True)
        nc.tensor.matmul(ltpl, g["w1t"], tbsb[:],
                         start=False, stop=False, skip_group_check=True)
        nc.tensor.matmul(ltpr, g["w0t"], gsb[:, P:2 * P],
                         start=False, stop=False, skip_group_check=True)
        nc.tensor.matmul(ltpr, g["w1t"], gsb[:, 2 * P:3 * P],
                         start=False, stop=True, skip_group_check=True)
        nc.scalar.copy(g["ar_in"][:], ltps[:])

    # ---------------- AllReduce (kv computed during the wait) ----------
    nc.sync.dma_start(out=g["cc_in"][:], in_=g["ar_in"][:])
    if getattr(nc, "_single_core_variant", False):
        nc.sync.dma_start(out=g["cc_out"][:], in_=g["cc_in"][:])
    else:
        nc.gpsimd.collective_compute(
            "AllReduce",
            mybir.AluOpType.add,
            ins=[g["cc_in"][:].opt()],
            outs=[g["cc_out"][:].opt()],
            replica_groups=REPLICA_GROUPS,
        )

    with tc.tile_pool(name="psK", bufs=3, space="PSUM") as psK:
        for bpair in range(NBLK // 2):
            kvp = psK.tile([P, 2 * BLK], F32, tag="kvp", name="kvp")
            for i, b in enumerate((2 * bpair, 2 * bpair + 1)):
                hs = slice(i * BLK, (i + 1) * BLK)
                bs = slice(b * BLK, (b + 1) * BLK)
                nc.tensor.matmul(kvp[:, hs], g["w0t"], xl[:, bs],
                                 start=True, stop=False)
            for i, b in enumerate((2 * bpair, 2 * bpair + 1)):
                hs = slice(i * BLK, (i + 1) * BLK)
                bs = slice(b * BLK, (b + 1) * BLK)
                nc.tensor.matmul(kvp[:, hs], g["w1t"], xr[:, bs],
                                 start=False, stop=True)
            # kv' = kv + bias folded into the evacuation
            for i, b in enumerate((2 * bpair, 2 * bpair + 1)):
                hs = slice(i * BLK, (i + 1) * BLK)
                bs = slice(b * BLK, (b + 1) * BLK)
                if b % 2 == 0:
                    nc.scalar.activation(
                        kv[:, bs], kvp[:, hs],
                        mybir.ActivationFunctionType.Identity,
                        bias=g["kvb"])
                else:
                    nc.vector.tensor_scalar_add(kv[:, bs], kvp[:, hs],
                                                g["kvb"])

    nc.sync.dma_start(out=g["ar_out"][:], in_=g["cc_out"][:])
    _post_ar(nc, tc, g, ol_d, or_d)


def _post_ar(nc, tc, g, ol_d, or_d):
    xl, xr, kv = g["xl"], g["xr"], g["kv"]
    ident, identr, ar_out = g["ident"], g["identr"], g["ar_out"]

    with (
        tc.tile_pool(name="psB", bufs=4, space="PSUM") as psB,
        tc.tile_pool(name="psS", bufs=3, space="PSUM") as psS,
        tc.tile_pool(name="sbM", bufs=1) as sbM,
        tc.tile_pool(name="sbStg", bufs=6) as sbStg,
    ):
        sides = []
        for s, (xres, t1) in enumerate([(xl, g["t1l"]), (xr, g["t1r"])]):
            # logits[c, k] = AR'd ltp^T + t1 (t1 = xsum_g x bias, host)
            lt2 = psS.tile([P, P], BF16, tag="smallps", name="lt2")
            nc.tensor.transpose(lt2[:], ar_out[:, s * P:(s + 1) * P],
                                identr[:])
            logits = sbM.tile([P, P], BF16, tag=f"logits{s}", name="logits")
            nc.vector.tensor_tensor(out=logits[:], in0=lt2[:], in1=t1,
                                    op=mybir.AluOpType.add)

            exps = sbM.tile([P, P], BF16, tag=f"exps{s}", name="exps")
            rsum = sbM.tile([P, 1], F32, tag=f"rsum{s}", name="rsum")
            recip = sbM.tile([P, 1], F32, tag=f"recip{s}", name="recip")
            expsT = sbM.tile([P, P], BF16, tag=f"expsT{s}", name="expsT")

            # exps = exp(logits*SM); logits*SM is O(1), no max-subtract
            # needed; rsum = row-sum(exps)
            nc.scalar.activation(exps[:], logits[:],
                                 mybir.ActivationFunctionType.Exp,
                                 scale=SM_SCALE,
                                 accum_out=rsum[:])
            nc.vector.reciprocal(recip[:], rsum[:])
            etp = psS.tile([P, P], BF16, tag="smallps", name="etp")
            nc.tensor.transpose(etp[:], exps[:], identr[:])
            nc.scalar.copy(expsT[:], etp[:])

            # ECA gate: glin = bt.T @ (recip .* (exps @ kvsum));
            # gate = 1/(1+exp(-glin - hb)) with hb host-computed.
            gs0 = psS.tile([P, 1], F32, tag="smallps", name="gs0")
            nc.tensor.matmul(gs0[:], expsT[:], g["kvs"],
                             start=True, stop=True)
            gsum = sbM.tile([P, 1], F32, tag=f"gsum{s}", name="gsum")
            nc.vector.tensor_scalar_mul(gsum[:], gs0[:], recip[:])
            glp = psS.tile([P, 1], F32, tag="smallps", name="glp")
            nc.tensor.matmul(glp[:], g["bt"], gsum[:],
                             start=True, stop=True)
            ge = sbM.tile([P, 1], F32, tag=f"ge{s}", name="ge")
            nc.scalar.activation(ge[:], glp[:],
                                 mybir.ActivationFunctionType.Exp,
                                 bias=(g["mhbl"] if s == 0 else g["mhbr"]), scale=-1.0)
            gep = sbM.tile([P, 1], F32, tag=f"gep{s}", name="gep")
            nc.vector.tensor_scalar_add(gep[:], ge[:], 1.0)
            gate = sbM.tile([P, 1], F32, tag=f"gate{s}", name="gate")
            nc.vector.reciprocal(gate[:], gep[:])
            combo = sbM.tile([P, 1], F32, tag=f"combo{s}", name="combo")
            nc.vector.tensor_scalar_mul(combo[:], recip[:], gate[:])

            # fold combo into the attention stationary
            exps_s = sbM.tile([P, P], BF16, tag=f"exps_s{s}", name="exps_s")
            nc.vector.tensor_scalar_mul(exps_s[:], exps[:], combo[:])
            etp2 = psS.tile([P, P], BF16, tag="smallps", name="etp2")
            nc.tensor.transpose(etp2[:], exps_s[:], identr[:])
            expsTs = sbM.tile([P, P], BF16, tag=f"expsTs{s}", name="expsTs")
            nc.vector.tensor_copy(expsTs[:], etp2[:])
            diag_g = sbM.tile([P, P], BF16, tag=f"diag{s}", name="diag_g")
            nc.vector.tensor_scalar_mul(diag_g[:], identr[:], gate[:])
            sides.append((xres, expsTs, gate, diag_g))

        # ---------------- Phase B ----------------
        # out = x*gate + exps_s @ kv'  (== gate.(attn@kv_biased + x))
        STG = 4 * BLK
        for s, (xres, expsTs, gate, diag_g) in enumerate(sides):
            out_d = ol_d if s == 0 else or_d
            for gi in range(NLOC // STG):
                stg = sbStg.tile([P, STG], BF16, tag="stg", name="stg")
                for k in range(STG // BLK):
                    b = gi * (STG // BLK) + k
                    bs = slice(b * BLK, (b + 1) * BLK)
                    ks = slice(k * BLK, (k + 1) * BLK)
                    ep = psB.tile([P, BLK], F32, tag="ep", name="ep")
                    if (2 * s + b) % 2 == 1:
                        nc.tensor.matmul(ep[:], expsTs[:], kv[:, bs],
                                         start=True, stop=False)
                        nc.tensor.matmul(ep[:], diag_g[:], xres[:, bs],
                                         start=False, stop=True)
                        nc.scalar.copy(stg[:, ks], ep[:])
                    else:
                        nc.tensor.matmul(ep[:], expsTs[:], kv[:, bs],
                                         start=True, stop=True)
                        nc.vector.scalar_tensor_tensor(
                            out=stg[:, ks], in0=xres[:, bs],
                            scalar=gate[:], in1=ep[:],
                            op0=mybir.AluOpType.mult,
                            op1=mybir.AluOpType.add)
                nc.sync.dma_start(out=out_d[:, gi * STG:(gi + 1) * STG],
                                  in_=stg[:])


def _get_nc(repeat: int = 1):
    if repeat not in _CACHE:
        _CACHE[repeat] = _build_nc(repeat)
    return _CACHE[repeat]


def _band_matrix(eca_w: np.ndarray) -> np.ndarray:
    """bt[i, j] = eca_w[i - j + 2] / N_GLOBAL  (zero outside the band).

    gate_lin = B @ gsum with B[c, c'] = w[c' - c + 2]; matmul computes
    lhsT.T @ rhs so we ship B.T, with the 1/N mean folded in.
    """
    k = eca_w.shape[0]
    assert k == 5
    bt = np.zeros((P, P), np.float32)
    for i in range(P):
        for j in range(max(0, i - 2), min(P, i + 3)):
            bt[i, j] = eca_w[i - j + 2]
    return bt / np.float32(N_GLOBAL)


def make_in_maps(left_feat, right_feat, kv_w, kv_b, eca_w):
    b, c, d, w, h = left_feat.shape
    assert (b, c, d * w * h) == (2, P, N_GLOBAL)
    lf = np.ascontiguousarray(left_feat, np.float32).reshape(b, c, -1)
    rf = np.ascontiguousarray(np.flip(right_feat, axis=-1),
                              dtype=np.float32).reshape(b, c, -1)
    kvw = np.asarray(kv_w, np.float32)                       # [128, 256]
    kvwT = np.ascontiguousarray(kvw.T)                       # [256, 128]
    w0t = kvwT[:P].astype(NPBF16)
    w1t = np.ascontiguousarray(kvwT[P:]).astype(NPBF16)
    kvbf = np.asarray(kv_b, np.float32).reshape(P)
    kvb = kvbf.reshape(P, 1).astype(np.float32)
    bt = _band_matrix(np.asarray(eca_w, np.float32))

    # host-side reductions (pure functions of the inputs)
    xsum = {}
    for bi in range(b):
        xsum[(bi, 0)] = lf[bi].sum(axis=1)                   # [128]
        xsum[(bi, 1)] = rf[bi].sum(axis=1)
    lfb = lf.astype(NPBF16)
    rfb = rf.astype(NPBF16)

    in_maps = []
    for core in range(N_CORES):
        bi, j = divmod(core, N_SPLIT)
        ns = slice(j * NLOC, (j + 1) * NLOC)
        xs_l, xs_r = xsum[(bi, 0)], xsum[(bi, 1)]
        # kvsum_g = W @ concat(xsum) + N*bias  (ECA kv-mean vector)
        kvs = (kvw[:, :P] @ xs_l + kvw[:, P:] @ xs_r
               + N_GLOBAL * kvbf).astype(np.float32)
        # rank-1 logit bias terms, added post-AR: t1_s = xsum_s x bias
        t1l = np.outer(xs_l, kvbf).astype(NPBF16)
        t1r = np.outer(xs_r, kvbf).astype(NPBF16)
        # ECA input-mean conv term, folded into the sigmoid bias:
        # gate = sigmoid(B@attn_part + hb), hb_s = bt.T @ xsum_s
        mhb = np.stack([-(bt.T @ xs_l), -(bt.T @ xs_r)],
                       axis=1).astype(np.float32)             # [128, 2]
        wpk = np.concatenate(
            [w0t, w1t, t1l, t1r, kvs.reshape(P, 1).astype(NPBF16)],
            axis=1)
        fpk = np.concatenate([bt, kvb, mhb], axis=1).astype(np.float32)
        in_maps.append({
            "xl": np.ascontiguousarray(lfb[bi, :, ns]),
            "xr": np.ascontiguousarray(rfb[bi, :, ns]),
            "wpk": np.ascontiguousarray(wpk),
            "fpk": np.ascontiguousarray(fpk),
        })
    return in_maps


def assemble(results, shape):
    b, c, d, w, h = shape
    enh_l = np.empty((b, c, N_GLOBAL), np.float32)
    enh_r = np.empty((b, c, N_GLOBAL), np.float32)
    for core in range(N_CORES):
        bi, j = divmod(core, N_SPLIT)
        ns = slice(j * NLOC, (j + 1) * NLOC)
        enh_l[bi, :, ns] = results[core]["ol"].astype(np.float32)
        enh_r[bi, :, ns] = results[core]["orr"].astype(np.float32)
    enh_l = enh_l.reshape(shape)
    enh_r = np.flip(enh_r.reshape(shape), axis=-1)
    return enh_l, enh_r


def run(in_maps, trace=False, **kw):
    nc = _get_nc()
    return run_bass_kernel_spmd(nc, in_maps, core_ids=list(range(N_CORES)),
                                trace=trace, **kw)


def kernel(left_feat, right_feat, kv_w, kv_b, eca_w):
    in_maps = make_in_maps(np.asarray(left_feat), np.asarray(right_feat),
                           np.asarray(kv_w), np.asarray(kv_b),
                           np.asarray(eca_w))
    res = run(in_maps)
    return assemble(res.results, np.asarray(left_feat).shape)
